# revision 35
# baseline (speedup 1.0000x reference)
"""nn_MultiHeadAttention (B=2, S=2048, D=2048, H=16) on 8 NeuronCores.

The reference module splits heads with a plain reshape (no transpose):
    Q = (x @ Wq.T).reshape(B, H, S, Dh)
so head h attends over ROWS [128h, 128h+128) of Qmat = x @ Wq.T, with
attention position s' = 16a + r mapping to (row 128h + a, feature slice
[128r, 128r+128)).  The merge DOES transpose (standard), so
    y = sum_h outh @ Wo[:, 128h:128h+128].T.

Sharding: core c handles batch b=c//4 and head-group g=c%4 (heads
4g..4g+3, i.e. tokens [512g, 512g+512) of its batch).  Each core
computes those projection row-slices against the FULL Wq/Wk/Wv, causal
attention in the scrambled index space, and a partial output projection
against its column slice of Wo.  The host sums the 4 partials per batch
in fp32 and unscrambles the column order.

Precision strategy (validated numerically, rel-err ~2.5e-3 vs fp32):
  * Q/K projections run in fp8e4m3 with DoubleRow perf mode (two
    128-partition contraction subtiles per matmul = 2x PE throughput).
    Scales: x*32, W*1024 (both < 240 max-normal), descaled 2^-15 at the
    PSUM->SBUF copy.  Softmax forgives the ~2.6% Q/K quantization.
  * Everything else runs fp16 (NOT bf16): same PE speed, 4x lower
    rounding error, and 2x/4x DVE throughput for the elementwise work.
  * V path / attention weights / output projection must NOT be fp8
    (measured 2.4e-2..3.8e-2 rel-err = over the 2e-2 gate).

Layout: projections stored as single tiles [dh=128, r=16, 512 tokens]
filled by ONE copy per 512-wide PSUM stripe (no per-head scatter).
Head hl's tiles are column slices [:, :, 128*hl : 128*hl+128].  Scores
use k-octets (free index i = 8r' + a'') against 512-wide q-blocks
(j = 32r + a_rel); causal masks precomputed on host for this order.
Softmax denominators: DVE accumulates the exp'd octets per q-block
(fp16, 4x mode), then a single ones-matmul per q-block broadcasts the
partition sum - removing ~30us of ones-matmuls from the PE stream.
Per-q-block finalize (last attn@V pair, ones-matmul, reciprocal,
normalize) is deferred until the next q-block's first score pair so the
PE never stalls on the DVE chain.  exp runs on 2-octet batches to halve
the activation-engine instruction overhead (ACT is the phase-B
co-bottleneck).  The output projection reuses phase-B PSUM tiles and
streams each [128,512] block to DRAM as it completes.
"""

import sys

try:
    import concourse.bass as bass
except ImportError:  # harness may not have the repo on PYTHONPATH
    for p in ("/root/.axon_site", "/root/.axon_site/_ro/trn_rl_repo",
              "/root/.axon_site/_ro/pypackages", "/opt/trn_rl_repo"):
        if p not in sys.path:
            sys.path.append(p)
    import concourse.bass as bass

import numpy as np

import concourse.mybir as mybir
import concourse.tile as tile
from concourse.bass_utils import run_bass_kernel_spmd

F32 = mybir.dt.float32
F16 = mybir.dt.float16
F8 = mybir.dt.float8e4
AF = mybir.ActivationFunctionType
DR = mybir.MatmulPerfMode.DoubleRow

B = 2
S = 2048
DM = 2048
H = 16
DH = 128
N_CORES = 8
HPC = 4                 # heads per core
DL = HPC * DH           # 512: per-core token-slice width
P = 128
QB = 512                # q-block width = 32 a x 16 r
N_DM = DM // P          # 16 contraction subtiles
NR = 16                 # r-stripes per head

SX = 32.0               # fp8 scale on x       (|x|max ~5.3  -> ~170 < 240)
SW = 1024.0             # fp8 scale on Wq/Wk   (|W|max ~0.12 -> ~120 < 240)
DESCALE = 1.0 / (SX * SW)


def _split_multi_waits(nc):
    """This container's walrus rejects >1 sync-wait per instruction.
    Hoist extra waits onto same-engine NoOps inserted just before."""
    ctr = 0
    for f in nc.m.functions:
        for bb in f.blocks:
            insts = bb.instructions
            fixes = []
            for idx, inst in enumerate(insts):
                si = inst.sync_info
                ow = list(si.on_wait) if si and si.on_wait else []
                if len(ow) > 1:
                    fixes.append((idx, inst, ow, si))
            for idx, inst, ow, si in reversed(fixes):
                inst.sync_info = mybir.SyncInfo(on_wait=ow[-1:], on_update=si.on_update)
                for w in reversed(ow[:-1]):
                    ctr += 1
                    nop = mybir.InstNoOp(
                        name=f"I-waitsplit-{ctr}", engine=inst.engine, ins=[], outs=[]
                    )
                    nop.sync_info = mybir.SyncInfo(on_wait=[w], on_update=[])
                    nc.register_instruction(nop, overwrite=True)
                    insts.insert(idx, nop)
    return ctr


def _build_nc():
    nc = bass.Bass(target_bir_lowering=False)

    # all inputs are pre-tiled on the host into the exact SBUF layouts so
    # every DMA is a contiguous per-partition run (strided weight-stripe
    # gathers cost ~1.7us of descriptor generation per issue on the sync
    # engine, which rate-limited phase A)
    x8_d = nc.dram_tensor("x8", [P, N_DM, DL], F8, kind="ExternalInput")
    x16_d = nc.dram_tensor("x16", [P, N_DM, DL], F16, kind="ExternalInput")
    wq8_d = nc.dram_tensor("wq8", [NR // 2, P, 2, N_DM, P], F8,
                           kind="ExternalInput")
    wk8_d = nc.dram_tensor("wk8", [NR // 2, P, 2, N_DM, P], F8,
                           kind="ExternalInput")
    wv_d = nc.dram_tensor("wv16", [NR // 2, P, 2, N_DM, P], F16,
                          kind="ExternalInput")
    wot_d = nc.dram_tensor("wot16", [P, HPC, DM], F16, kind="ExternalInput")
    mask_d = nc.dram_tensor("maskc", [P, 4, QB], F16, kind="ExternalInput")
    ones_d = nc.dram_tensor("ones", [P, P], F16, kind="ExternalInput")
    ident_d = nc.dram_tensor("ident", [P, P], F16, kind="ExternalInput")
    yt_d = nc.dram_tensor("yt", [DM, S], F16, kind="ExternalOutput")     # partial y[b].T

    yt_t3 = yt_d.rearrange("(o p) s -> p o s", p=P)

    with tile.TileContext(nc) as tc:
        with (
            tc.tile_pool(name="proj", bufs=1) as proj,
            tc.tile_pool(name="bconst", bufs=1) as bconst,
            tc.tile_pool(name="stg", bufs=3) as stg,
            tc.tile_pool(name="atp", bufs=1) as atp,
            tc.tile_pool(name="accp", bufs=12) as accp,
            tc.tile_pool(name="rcp", bufs=2) as rcp,
            nc.allow_low_precision(reason="fp8/fp16 attention kernel"),
        ):
            # projection tiles [dh, token, r]; head hl = cols [128hl, 128hl+128)
            qt = proj.tile([P, DL, NR], F16, tag="qt")
            kt = proj.tile([P, DL, NR], F16, tag="kt")
            vt = proj.tile([P, DL, NR], F16, tag="vt")
            ones_t = bconst.tile([P, P], F16, tag="ones")
            ident_t = bconst.tile([P, P], F16, tag="ident")
            mask_t = bconst.tile([P, 4, QB], F16, tag="mask")

            # score-pass: scores + exp + causal masks + denominator adds for
            # one (head, q-block).  Shared by the phase-A prepass (head 0,
            # hiding its exp work under the V projection) and phase B.
            def emit_score_pass(hl, qb, at, acc_a, acc_b, pspool,
                                pair_hook=None):
                nk = 4 * qb + 4
                npair = nk // 2
                a0 = 32 * qb
                c0 = hl * P
                for t in range(npair):
                    # the last pair holds diagonal octets 4qb+2/4qb+3 whose
                    # valid q-columns are [256, 512): compute only those,
                    # zero the dead half explicitly
                    rstr = t == npair - 1
                    lo = 256 if rstr else 0
                    if rstr:
                        nc.gpsimd.memset(at[:, 2 * t:2 * t + 2, :lo], 0)
                    ps2 = pspool.tile([P, 2, QB], F32, tag="ps2")
                    for u in (2 * t, 2 * t + 1):
                        nc.tensor.matmul(
                            ps2[:, u - 2 * t, lo:],
                            lhsT=kt[:, c0 + 8 * u:c0 + 8 * u + 8, :],
                            rhs=qt[:, c0 + a0 + lo // 16:c0 + a0 + 32, :],
                            start=True, stop=True,
                        )
                    if pair_hook is not None:
                        pair_hook(t)
                    nc.scalar.activation(
                        at[:, 2 * t:2 * t + 2, lo:], ps2[:, :, lo:],
                        AF.Exp, scale=1.0 / DH,
                    )
                    for u in (2 * t, 2 * t + 1):
                        if u >= 4 * qb:
                            # causal mask: only columns [128d, 128d+128) are
                            # partial; below them at must be zero, above them
                            # the mask is all-ones
                            dd = u - 4 * qb
                            ms = 128 * dd
                            if dd == 1:
                                nc.gpsimd.memset(at[:, u, 0:128], 0)
                            elif dd == 3:
                                nc.gpsimd.memset(at[:, u, 256:384], 0)
                            nc.gpsimd.tensor_mul(
                                at[:, u, ms:ms + 128],
                                at[:, u, ms:ms + 128],
                                mask_t[:, dd, ms:ms + 128],
                            )
                    # ping-pong accumulator (in-place adds run 1x on DVE)
                    if t == 0:
                        nc.vector.tensor_add(acc_a[:], at[:, 0, :],
                                             at[:, 1, :])
                    else:
                        nc.vector.tensor_add(acc_b[:], acc_a[:],
                                             at[:, 2 * t, :])
                        nc.vector.tensor_add(acc_a[:], acc_b[:],
                                             at[:, 2 * t + 1, :])

            def alloc_qb2(hl, qb):
                nk = 4 * qb + 4
                tag = f"at{qb}b" if (hl, qb) == (1, 3) else f"at{qb}"
                at = atp.tile([P, nk, QB], F16, tag=tag,
                              name=f"at_h{hl}_{qb}")
                acc_a = accp.tile([P, QB], F16, tag="acc",
                                  name=f"acca_h{hl}_{qb}")
                acc_b = accp.tile([P, QB], F16, tag="acc",
                                  name=f"accb_h{hl}_{qb}")
                return at, acc_a, acc_b

            # ---- phase A: projections straight into SBUF ----
            pre = {}   # head-0 prepass tiles, consumed by phase B
            with (
                tc.tile_pool(name="xp", bufs=1) as xp,
                tc.tile_pool(name="wp", bufs=3) as wp,
                tc.tile_pool(name="ps_a", bufs=2, space="PSUM") as ps_a,
                tc.tile_pool(name="ps2a", bufs=2, space="PSUM") as ps2a,
            ):
                x8_t = xp.tile([P, N_DM, DL], F8, tag="x8")
                x16_t = xp.tile([P, N_DM, DL], F16, tag="x16")
                nc.sync.dma_start(x8_t[:, 0:4, :], x8_d[:, 0:4, :])
                nc.sync.dma_start(x8_t[:, 4:8, :], x8_d[:, 4:8, :])

                for w_i, (w_d, w_dt, dst) in enumerate((
                    (wq8_d, F8, qt),
                    (wk8_d, F8, kt),
                    (wv_d, F16, vt),
                )):
                    if w_i == 1:
                        # constants stream during the K phase; the head-0
                        # prepass needs the masks right after K
                        nc.sync.dma_start(ident_t[:], ident_d[:])
                        nc.sync.dma_start(ones_t[:], ones_d[:])
                        nc.sync.dma_start(mask_t[:], mask_d[:])
                    if w_i == 2:
                        # score prepass for head 0 (all q-blocks) and head
                        # 1's largest q-block: the scores ride the PE between
                        # K and V; the exp/mask/add chains hide under the
                        # 56us of V-projection matmuls that follow
                        for phl, pqb in ((0, 0), (0, 1), (0, 2), (0, 3),
                                         (1, 3)):
                            key = (phl, pqb)
                            pre[key] = alloc_qb2(phl, pqb)
                            emit_score_pass(phl, pqb, *pre[key], ps2a)
                    for rp in range(NR // 2):
                        if w_i == 1 and rp < 4:
                            # x16 streamed in chunks between K stripes so it
                            # doesn't stall the K weight stream
                            nc.sync.dma_start(
                                x16_t[:, 4 * rp:4 * rp + 4, :],
                                x16_d[:, 4 * rp:4 * rp + 4, :],
                            )
                        psum = ps_a.tile([P, 2, QB], F32, tag="pa")
                        # one DMA per pre-tiled stripe pair: strided gathers
                        # cost ~1.7us of descriptor generation per issue and
                        # rate-limited stripe delivery
                        w_t = wp.tile([P, 2, N_DM, P], w_dt, tag=f"w{w_i}")
                        nc.sync.dma_start(w_t[:], w_d[rp])
                        if w_i == 0 and rp == 0:
                            # x8 upper half rides behind the first Q stripes
                            nc.sync.dma_start(
                                x8_t[:, 8:12, :], x8_d[:, 8:12, :]
                            )
                            nc.sync.dma_start(
                                x8_t[:, 12:16, :], x8_d[:, 12:16, :]
                            )
                        for half in range(2):
                            if w_dt == F8:
                                for d in range(8):
                                    nc.tensor.matmul(
                                        psum[:, half, :],
                                        lhsT=w_t[:, half, 2 * d:2 * d + 2, :],
                                        rhs=x8_t[:, 2 * d:2 * d + 2, :],
                                        start=(d == 0), stop=(d == 7),
                                        perf_mode=DR,
                                    )
                            else:
                                for d in range(N_DM):
                                    nc.tensor.matmul(
                                        psum[:, half, :],
                                        lhsT=w_t[:, half, d, :],
                                        rhs=x16_t[:, d, :],
                                        start=(d == 0), stop=(d == N_DM - 1),
                                    )
                        # one paired scatter copy (4-byte token units; the
                        # 2-byte strided write pattern measured 3x slower),
                        # spread across engines
                        dst_ap = dst[:, :, 2 * rp:2 * rp + 2]
                        src_ap = psum[:].rearrange("p t c -> p c t")
                        if w_i == 0 or (w_i == 2 and rp % 2 == 0):
                            nc.vector.tensor_scalar_mul(
                                dst_ap, src_ap, DESCALE if w_i == 0 else 1.0
                            )
                        elif w_i == 1:
                            nc.scalar.mul(dst_ap, src_ap, DESCALE)
                        else:
                            nc.scalar.copy(dst_ap, src_ap)

            # ---- phase B + C ----
            with (
                tc.tile_pool(name="attp", bufs=HPC) as attp,
                tc.tile_pool(name="wop", bufs=1) as wop,
                tc.tile_pool(name="vkp", bufs=2) as vkp,
                tc.tile_pool(name="ps2", bufs=2, space="PSUM") as ps2p,
                tc.tile_pool(name="ps_o", bufs=2, space="PSUM") as ps_op,
                tc.tile_pool(name="ps_l", bufs=1, space="PSUM") as ps_lp,
                tc.tile_pool(name="ps_t", bufs=1, space="PSUM") as ps_tp,
            ):
                # normalized attention outputs per head [dh, qb, j]
                att = [attp.tile([P, 4, QB], F16, tag="att", name=f"att{i}")
                       for i in range(HPC)]
                wot_t = wop.tile([P, HPC, DM], F16, tag="wo")
                nc.sync.dma_start(wot_t[:], wot_d[:])

                pend_a = []   # deferred last attn@V pairs
                pend_b = []   # deferred denominator chains

                def flush_pending(q):
                    while q:
                        q.pop(0)()

                def emit_tr4(vk_dst, src_hl, m0):
                    # k-major V tiles via PE transpose: vk[i=16a''+r', m, dh].
                    # 4 transposes share one PSUM bank; one DVE copy drains it
                    # (GPSIMD cannot read PSUM on this target).
                    ps_t = ps_tp.tile([P, 4, P], F16, tag="pt")
                    for k in range(4):
                        cc = src_hl * P + 8 * (m0 + k)
                        nc.tensor.transpose(
                            ps_t[:, k, :], vt[:, cc:cc + 8, :], ident_t[:]
                        )
                    nc.vector.tensor_copy(vk_dst[:, m0:m0 + 4, :], ps_t[:])

                vk = vkp.tile([P, NR, P], F16, tag="vk", name="vk0")
                emit_tr4(vk, 0, 0)

                for hl in range(HPC):
                    vk_next = (vkp.tile([P, NR, P], F16, tag="vk",
                                        name=f"vk{hl + 1}")
                               if hl + 1 < HPC else None)

                    for qb in range(4):
                        nk = 4 * qb + 4
                        npair = nk // 2
                        prescored = (hl, qb) in pre
                        if prescored:
                            at, acc_a, acc_b = pre[(hl, qb)]
                        else:
                            at, acc_a, acc_b = alloc_qb2(hl, qb)
                        psum_o = ps_op.tile([P, QB], F32, tag="po")

                        def emit_av(u, vk=vk, at=at, psum_o=psum_o, nk=nk):
                            nc.tensor.matmul(
                                psum_o[:], lhsT=vk[:, u, :], rhs=at[:, u, :],
                                start=(u == 0), stop=(u == nk - 1),
                            )

                        def pair_hook(t, hl=hl, qb=qb, npair=npair,
                                      vk=vk, vk_next=vk_next,
                                      emit_av=emit_av):
                            # future vk transposes ride the score stream:
                            # qb0 preps octets 4..7, qb1 8..11, qb2 12..15,
                            # qb3 the next head's 0..3
                            if t == 1:
                                if qb < 3:
                                    emit_tr4(vk, hl, 4 * (qb + 1))
                                elif vk_next is not None:
                                    emit_tr4(vk_next, hl + 1, 0)
                            # previous q-block's finalize is split: the last
                            # attn@V pair flushes early (gated on fast
                            # ACT/Pool work), the denominator chain on the
                            # LAST pair so the DVE add-chain has a whole
                            # q-block of slack before the PE waits on it
                            if t == 1:
                                flush_pending(pend_a)
                            if t == npair - 1:
                                flush_pending(pend_b)
                            if t >= 2:
                                emit_av(2 * t - 4)
                                emit_av(2 * t - 3)

                        if prescored:
                            # prescored in phase A: only the attn@V side runs
                            for t in range(npair):
                                pair_hook(t)
                        else:
                            emit_score_pass(hl, qb, at, acc_a, acc_b,
                                            ps2p, pair_hook)
                        emit_av(nk - 4)
                        emit_av(nk - 3)

                        def fin_avs(nk=nk, emit_av=emit_av):
                            emit_av(nk - 2)
                            emit_av(nk - 1)

                        def fin_den(hl=hl, qb=qb, acc=acc_a, psum_o=psum_o):
                            psum_l = ps_lp.tile([P, QB], F32, tag="pl")
                            nc.tensor.matmul(
                                psum_l[:], lhsT=ones_t[:], rhs=acc[:],
                                start=True, stop=True,
                            )
                            rcb = rcp.tile([P, QB], F32, tag="rcb")
                            # 1/l = exp(-ln(l)) on the scalar engine: both
                            # funcs live in one ACT table (no reload thrash).
                            # |l| in [1, ~300] is in-domain; ~1e-3 rel err
                            # adds ~0.1% output error.  (A DVE InstReciprocal
                            # variant measured 3.35us and stalled the PE.)
                            nll = rcp.tile([P, QB], F16, tag="nll")
                            nc.scalar.activation(nll[:], psum_l[:], AF.Ln)
                            nc.scalar.activation(rcb[:], nll[:], AF.Exp,
                                                 scale=-1.0)
                            nc.vector.tensor_mul(
                                att[hl][:, qb, :], psum_o[:], rcb[:]
                            )

                        pend_a.append(fin_avs)
                        pend_b.append(fin_den)

                    vk = vk_next

                # ---- phase C: partial yT = WoT.T @ att, reusing B psum ----
                # sb pairs share one staging tile and one DMA (the per-issue
                # cost on the sync engine made 64 block-DMAs a bottleneck)
                for ot in range(N_DM):
                    for sp in range(2):
                        st = stg.tile([P, 2, QB], F16, tag="st")
                        for sh in range(2):
                            sb = 2 * sp + sh
                            pool = ps_op if (ot + sb) % 2 == 0 else ps_lp
                            psc = pool.tile([P, QB], F32,
                                            tag="po" if pool is ps_op else "pl")
                            for hl2 in range(HPC):
                                nc.tensor.matmul(
                                    psc[:],
                                    lhsT=wot_t[:, hl2, ot * P:(ot + 1) * P],
                                    rhs=att[hl2][:, sb, :],
                                    start=(hl2 == 0), stop=(hl2 == HPC - 1),
                                )
                            if ot == 0 and sb == 0:
                                flush_pending(pend_a)
                                flush_pending(pend_b)
                            if (ot + sb) % 2 == 0:
                                nc.vector.tensor_copy(st[:, sh, :], psc[:])
                            else:
                                nc.scalar.copy(st[:, sh, :], psc[:])
                        nc.sync.dma_start(
                            yt_t3[:, ot, 2 * sp * QB:(2 * sp + 2) * QB], st[:]
                        )

    _split_multi_waits(nc)
    return nc


_NC = None


def _make_masks():
    # causal masks for diagonal octets in (a-outer, r-inner) index order:
    # k partition i = 16a'' + r';  q column j = 16a_rel + r  (== position
    # within the q-block, so yt columns come out in plain s' order)
    # allow: 16*(8*delta + a'') + r' <= 16*a_rel + r
    k_lin = (16 * np.arange(8)[:, None] + np.arange(NR)[None, :]).reshape(-1)
    q_lin = (16 * np.arange(32)[:, None] + np.arange(NR)[None, :]).reshape(-1)
    out = np.empty((4, P, QB), dtype=np.float32)
    for d in range(4):
        out[d] = ((k_lin[:, None] + P * d) <= q_lin[None, :]).astype(np.float32)
    return out


def kernel(x, Wq, Wk, Wv, Wo, _want_trace=False, **_trace_kw):
    global _NC
    if _NC is None:
        _NC = _build_nc()
    nc = _NC

    import ml_dtypes
    f8 = ml_dtypes.float8_e4m3
    f16 = np.float16

    x = np.asarray(x, dtype=np.float32)
    Wq = np.asarray(Wq, dtype=np.float32)
    Wk = np.asarray(Wk, dtype=np.float32)
    Wv = np.asarray(Wv, dtype=np.float32)
    Wo = np.asarray(Wo, dtype=np.float32)

    def tile_w(WT, dt, scale=1.0):
        # [row=o*128+p, col=(2rp+h)*128+c] -> [rp, p, h, o, c]
        a = WT * scale if scale != 1.0 else WT
        a = a.reshape(N_DM, P, NR // 2, 2, P).transpose(2, 1, 3, 0, 4)
        return np.ascontiguousarray(a).astype(dt)

    wq8 = tile_w(Wq.T, f8, SW)
    wk8 = tile_w(Wk.T, f8, SW)
    wv16 = tile_w(Wv.T, f16)
    masks = np.ascontiguousarray(
        _make_masks().transpose(1, 0, 2)).astype(f16)
    ones = np.ones((P, P), dtype=f16)
    ident = np.eye(P, dtype=np.float32).astype(f16)

    def tile_x(xs):  # [row=o*128+p, s] -> [p, o, s]
        return np.ascontiguousarray(
            xs.reshape(N_DM, P, DL).transpose(1, 0, 2))

    in_maps = []
    for c in range(N_CORES):
        b, g = divmod(c, HPC)
        sl = slice(g * DL, (g + 1) * DL)
        xs = x[b, sl, :].T
        wot = Wo[:, sl].T.reshape(HPC, P, DM).transpose(1, 0, 2)
        in_maps.append({
            "x8": tile_x((xs * SX).astype(np.float32)).astype(f8),
            "x16": tile_x(xs).astype(f16),
            "wq8": wq8,
            "wk8": wk8,
            "wv16": wv16,
            "wot16": np.ascontiguousarray(wot).astype(f16),
            "maskc": masks,
            "ones": ones,
            "ident": ident,
        })

    res = run_bass_kernel_spmd(
        nc, in_maps, list(range(N_CORES)),
        trace=_want_trace, **_trace_kw,
    )

    y = np.empty((B, S, DM), dtype=np.float32)
    for b in range(B):
        acc = res.results[HPC * b]["yt"].astype(np.float32)
        for g in range(1, HPC):
            acc += res.results[HPC * b + g]["yt"].astype(np.float32)
        y[b] = acc.T
    if _want_trace:
        return y, res
    return y
